# revision 26
# baseline (speedup 1.0000x reference)
"""
DiffusionFlowEmbedder loss kernel for 8 Trainium2 NeuronCores.

Math (n=4096, d=100, e=2):
  P = exp((flows @ X.T - c'_i)/sigma),  c'_i = ||f_i||^2 + x_i.f_i   (n x n)
  loss = sum(P^4 * (log P^4 - log Pemb^4))/n + mean((recon - X)^2)
  where Pemb = rownorm(exp affinity of the encoder embedding).

Strategy: row-shard the n x n work over 8 cores with ZERO collectives.
Each core owns 512 rows and computes T_s = (M^s [rows,:])^T via the
recurrence T_{s+1} = matmul(lhsT=M, rhs=T_s) (out = M^T @ T_s), where the
full M (P_graph or A_emb) is generated on-device once (affinity matmul +
exp on the scalar engine) into DRAM bf16 and streamed back as the
stationary operand. Affinity row constants (-2c') ride the contraction
as an augmentation row; row-normalization of A is folded into exp biases
(-log Z) and per-partition PSUM->SBUF copy scales (1/Z), with Z obtained
free via the activation accum_out. The per-core output is three partial
sum vectors; the host reduces them to the final scalar.
"""

import numpy as np
import ml_dtypes

import concourse.bass as bass
import concourse.mybir as mybir
import concourse.tile as tile
from concourse import bacc
from concourse.bass_utils import run_bass_kernel_spmd

F32 = mybir.dt.float32
BF16 = mybir.dt.bfloat16
AF = mybir.ActivationFunctionType
OP = mybir.AluOpType

NCORES = 8
N = 4096
D = 100
E = 2
R = N // NCORES          # 512 rows per core
NT = N // 128            # 32 partition tiles
RT = R // 128            # 4
ISIG = 2.0               # 1/sigma
C0 = 8.3125              # ~log(4096), exact in bf16; centers the -logZr row

WSHAPES = [
    ("enc1", [D, D]), ("enc2", [D, 10]), ("enc3", [10, E]),
    ("fa1", [E, 10]), ("fa2", [10, 10]), ("fa3", [10, E]),
    ("dec1", [E, 10]), ("dec2", [10, D]), ("dec3", [D, D]),
]


def _emit(ctx, tc, IN, OUT, stop="full"):
    nc = tc.nc

    def _stop_here(name):
        if stop != name:
            return False
        nc.sync.dma_start(OUT["acc1_out"].ap(), acc1)
        nc.sync.dma_start(OUT["acc2_out"].ap(), acc2)
        return True

    pers = ctx.enter_context(tc.tile_pool(name="pers", bufs=1))
    stages = ctx.enter_context(tc.tile_pool(name="stages", bufs=3))
    lpool = ctx.enter_context(tc.tile_pool(name="lpool", bufs=2))
    zpool = ctx.enter_context(tc.tile_pool(name="zpool", bufs=4))
    crows = ctx.enter_context(tc.tile_pool(name="crows", bufs=2))
    chain = ctx.enter_context(tc.tile_pool(name="chain", bufs=3))
    dram = ctx.enter_context(tc.tile_pool(name="dram", bufs=1, space="DRAM"))
    pc = ctx.enter_context(tc.tile_pool(name="pc", bufs=3, space="PSUM"))
    pg = ctx.enter_context(tc.tile_pool(name="pg", bufs=2, space="PSUM"))
    ps = ctx.enter_context(tc.tile_pool(name="ps", bufs=1, space="PSUM"))

    # strip-major layout: [m-group, partition, k-tile, m-within-group] so a
    # chain strip is one fully-contiguous-per-partition DMA read.
    P_d = dram.tile([NT // 2, 128, NT, 256], BF16, tag="P_d")
    A_d = dram.tile([NT // 2, 128, NT, 256], BF16, tag="A_d")

    def sbin(pool, name, shape, dtype=BF16):
        t = pool.tile(shape, dtype, tag=name)
        nc.sync.dma_start(t, IN[name].ap())
        return t

    # persistent small inputs
    xr = sbin(pers, "xrT_aug", [D + 1, R])
    fr = sbin(pers, "flowsrT", [D, R])       # pre-scaled 2*flows^T rows
    W = {}
    B = {}
    for name, (k, m) in WSHAPES:
        W[name] = sbin(pers, name, [k, m])
        B[name] = sbin(pers, name + "b", [m, 1], F32)

    ones = pers.tile([128, 1], BF16, tag="ones")
    nc.vector.memset(ones, 1.0)
    onesrow = pers.tile([1, N], BF16, tag="onesrow")
    nc.vector.memset(onesrow, 1.0)
    acc1 = pers.tile([128, 1], F32, tag="acc1")
    acc2 = pers.tile([128, 1], F32, tag="acc2")
    nc.vector.memset(acc1, 0.0)
    nc.vector.memset(acc2, 0.0)

    def linear(name, rhs, out, func):
        w = W[name]
        m = w.shape[-1]
        cols = rhs.shape[-1]
        psum = ps.tile([128, 512], F32, tag="ps")
        nc.tensor.matmul(psum[:m, :cols], lhsT=w, rhs=rhs, start=True, stop=True)
        nc.scalar.activation(out, psum[:m, :cols], func, bias=B[name])

    # long-lived mid tiles
    xe_aug = pers.tile([E + 1, N], BF16, tag="xe_aug")   # [emb^T; 1]
    nc.sync.dma_start(xe_aug[E : E + 1, :], onesrow)
    f_aug_e = pers.tile([E + 1, N], BF16, tag="f_aug_e")
    Z = pers.tile([128, NT], F32, tag="Z")
    nc.vector.memset(Z, 0.0)
    T0g = chain.tile([128, NT, 512], BF16, tag="T")
    T0e = chain.tile([128, NT, 512], BF16, tag="T")

    # =============== phase 1: prep (scoped tiles) ===============
    # host feeds flowsT/flowsrT pre-scaled by 2 (= 1/sigma): the input IS
    # the f_aug main block. c' sums then carry a 4x factor folded into the
    # -0.5 copy scale: (2x + 2f) . 2f = 4(x.f + f.f),  -2c' = -0.5 * that.
    with tc.tile_pool(name="ph1", bufs=1) as ph1:
        xt = sbin(ph1, "xT_aug", [D + 1, N])
        f_aug_g = ph1.tile([D + 1, N], BF16, tag="f_aug_g")
        nc.sync.dma_start(f_aug_g[:D, :], IN["flowsT"].ap())

        # ---- full-width MLPs (transposed layout, bias via ACT) ----
        h1a = ph1.tile([D, N], BF16, tag="tmpA")
        h2a = ph1.tile([10, N], BF16, tag="tmpB")
        for sl in range(N // 512):
            s = slice(sl * 512, (sl + 1) * 512)
            linear("enc1", xt[:D, s], h1a[:, s], AF.Relu)
            linear("enc2", h1a[:, s], h2a[:, s], AF.Relu)
            linear("enc3", h2a[:, s], xe_aug[:E, s], AF.Identity)
        g1a = ph1.tile([10, N], BF16, tag="tmpA")
        g2a = ph1.tile([10, N], BF16, tag="tmpB")
        for sl in range(N // 512):
            s = slice(sl * 512, (sl + 1) * 512)
            linear("fa1", xe_aug[:E, s], g1a[:, s], AF.Tanh)
            linear("fa2", g1a[:, s], g2a[:, s], AF.Tanh)
            # ef^T * 2 straight into f_aug_e rows (fa3 W,b pre-scaled by 2)
            linear("fa3", g2a[:, s], f_aug_e[:E, s], AF.Identity)

        # ---- affinity aug rows: -2c' staged at partition 0, DMA'd in ----
        sg = ph1.tile([D, N], BF16, tag="sgse")
        nc.vector.scalar_tensor_tensor(
            sg, in0=xt[:D, :], scalar=ISIG, in1=f_aug_g[:D, :],
            op0=OP.mult, op1=OP.add)
        nc.vector.tensor_mul(sg, sg, f_aug_g[:D, :])
        for sl in range(N // 512):
            s = slice(sl * 512, (sl + 1) * 512)
            psum = ps.tile([128, 512], F32, tag="ps")
            nc.tensor.matmul(psum[:1, :], lhsT=ones[:D, :], rhs=sg[:, s],
                             start=True, stop=True)
            cr = crows.tile([1, 512], BF16, tag="crow")
            nc.scalar.activation(cr, psum[:1, :], AF.Copy, scale=-0.5)
            nc.sync.dma_start(f_aug_g[D : D + 1, s], cr)

        se = ph1.tile([E, N], BF16, tag="sgse")
        nc.vector.scalar_tensor_tensor(
            se, in0=xe_aug[:E, :], scalar=ISIG, in1=f_aug_e[:E, :],
            op0=OP.mult, op1=OP.add)
        nc.vector.tensor_mul(se, se, f_aug_e[:E, :])
        for sl in range(N // 512):
            s = slice(sl * 512, (sl + 1) * 512)
            psum = ps.tile([128, 512], F32, tag="ps")
            nc.tensor.matmul(psum[:1, :], lhsT=ones[:E, :], rhs=se[:, s],
                             start=True, stop=True)
            cr = crows.tile([1, 512], BF16, tag="crow")
            nc.scalar.activation(cr, psum[:1, :], AF.Copy, scale=-0.5)
            nc.sync.dma_start(f_aug_e[E : E + 1, s], cr)

        # ---- per-core rows versions ----
        xer = pers.tile([E + 1, R], BF16, tag="xer")
        nc.sync.dma_start(xer[E : E + 1, :], onesrow[0:1, :R])
        h1r = pers.tile([D, R], BF16, tag="h1r")
        h2r = pers.tile([10, R], BF16, tag="h2r")
        f_aug_er = pers.tile([E + 1, R], BF16, tag="f_aug_er")
        linear("enc1", xr[:D, :], h1r, AF.Relu)
        linear("enc2", h1r, h2r, AF.Relu)
        linear("enc3", h2r, xer[:E, :], AF.Identity)
        linear("fa1", xer[:E, :], h2r, AF.Tanh)
        linear("fa2", h2r, h1r[:10, :], AF.Tanh)
        linear("fa3", h1r[:10, :], f_aug_er[:E, :], AF.Identity)

        # recon loss partial: dec MLP on emb rows
        linear("dec1", xer[:E, :], h2r, AF.Relu)
        linear("dec2", h2r, h1r, AF.Relu)
        psum = ps.tile([128, 512], F32, tag="ps")
        nc.tensor.matmul(psum[:D, :R], lhsT=W["dec3"], rhs=h1r, start=True,
                         stop=True)
        rec = lpool.tile([128, 512], F32, tag="ltmp")
        nc.scalar.activation(rec[:D, :R], psum[:D, :R], AF.Identity,
                             bias=B["dec3"])
        diff = lpool.tile([128, 512], F32, tag="ptmp")
        nc.vector.tensor_sub(diff[:D, :R], rec[:D, :R], xr[:D, :])
        sq = lpool.tile([128, 512], F32, tag="ltmp")
        racc = pers.tile([D, 1], F32, tag="racc")
        nc.scalar.activation(sq[:D, :R], diff[:D, :R], AF.Square,
                             accum_out=racc)
        nc.sync.dma_start(OUT["recon_out"].ap(), racc)

        # f_aug_gr / f_aug_er aug rows (-2c'); fr is pre-scaled 2*flows^T
        f_aug_gr = pers.tile([D + 1, R], BF16, tag="f_aug_gr")
        nc.vector.tensor_copy(f_aug_gr[:D, :], fr)
        sgr = pers.tile([D, R], BF16, tag="sgr")
        nc.vector.scalar_tensor_tensor(
            sgr, in0=xr[:D, :], scalar=ISIG, in1=fr, op0=OP.mult, op1=OP.add)
        nc.vector.tensor_mul(sgr, sgr, fr)
        psum = ps.tile([128, 512], F32, tag="ps")
        nc.tensor.matmul(psum[:1, :R], lhsT=ones[:D, :], rhs=sgr, start=True,
                         stop=True)
        cgr = crows.tile([1, 512], BF16, tag="crow")
        nc.scalar.activation(cgr, psum[:1, :R], AF.Copy, scale=-0.5)
        nc.sync.dma_start(f_aug_gr[D : D + 1, :], cgr)

        ser = pers.tile([E, R], BF16, tag="ser")
        nc.vector.scalar_tensor_tensor(
            ser, in0=xer[:E, :], scalar=ISIG, in1=f_aug_er[:E, :],
            op0=OP.mult, op1=OP.add)
        nc.vector.tensor_mul(ser, ser, f_aug_er[:E, :])
        psum = ps.tile([128, 512], F32, tag="ps")
        nc.tensor.matmul(psum[:1, :R], lhsT=ones[:E, :], rhs=ser, start=True,
                         stop=True)
        cer = pers.tile([1, R], BF16, tag="cer")   # keep: reused for er2
        nc.scalar.activation(cer, psum[:1, :R], AF.Copy, scale=-0.5)
        nc.sync.dma_start(f_aug_er[E : E + 1, :], cer)

        # ---- Z_rows pass: row sums of A[rows, :] ----
        Zr = pers.tile([128, RT], F32, tag="Zr")
        nc.vector.memset(Zr, 0.0)
        junk = pers.tile([128, 512], BF16, tag="junk")
        for it in range(RT):
            for jg in range(N // 512):
                psum = pg.tile([128, 1024], F32, tag="pg")
                nc.tensor.matmul(
                    psum[:, :512],
                    lhsT=f_aug_er[:, it * 128 : (it + 1) * 128],
                    rhs=xe_aug[:, jg * 512 : (jg + 1) * 512],
                    start=True, stop=True,
                )
                zt = zpool.tile([128, 1], F32, tag="zt")
                nc.scalar.activation(junk, psum[:, :512], AF.Exp, accum_out=zt)
                nc.vector.tensor_add(Zr[:, it : it + 1], Zr[:, it : it + 1], zt)

        # -log(Zr) shuffled from [128, RT] (partition-major) to a [1, R] row
        lzr = pers.tile([128, RT], F32, tag="lzr")
        nc.scalar.activation(lzr, Zr, AF.Ln)
        nc.vector.tensor_scalar_mul(lzr, lzr, -1.0)
        zrow = pers.tile([1, R], F32, tag="zrow")
        ztmp_d = dram.tile([128, RT], F32, tag="ztmp_d")
        nc.sync.dma_start(ztmp_d, lzr)
        with nc.allow_non_contiguous_dma(reason="512-elem partition->row shuffle"):
            nc.sync.dma_start(
                zrow.rearrange("one (t p) -> one t p", p=128),
                ztmp_d.rearrange("p t -> t p")[None],
            )
        # center the aug row near 0 (logZr ~ log(4096) ~ 8.318 would quantize
        # to bf16 step 0.0625 -> systematic T0e shift); C0 is re-added via the
        # fp32 ACT bias of the T0e exp below.
        f_aug_er2 = pers.tile([E + 1, R], BF16, tag="f_aug_er2")
        nc.vector.tensor_copy(f_aug_er2[:E, :], f_aug_er[:E, :])
        zrow2 = pers.tile([1, R], BF16, tag="zrow2")
        nc.vector.scalar_tensor_tensor(
            zrow2, in0=zrow, scalar=C0, in1=cer, op0=OP.add, op1=OP.add)
        nc.sync.dma_start(f_aug_er2[E : E + 1, :], zrow2)

        # ---- T0g: (P[rows,:])^T  [4096, 512] ----
        for jt in range(NT):
            psum = pg.tile([128, 1024], F32, tag="pg")
            nc.tensor.matmul(
                psum[:, :512],
                lhsT=xt[:, jt * 128 : (jt + 1) * 128],
                rhs=f_aug_gr,
                start=True, stop=True,
            )
            nc.scalar.activation(T0g[:, jt, :], psum[:, :512], AF.Exp)

        # ---- P generation (standalone; nothing else ready yet) ----
        def gen_units(f_aug, x_aug, dst, z_acc):
            for kt in range(NT):
                for mh in range(4):
                    psum = pg.tile([128, 1024], F32, tag="pg")
                    for h in range(2):
                        nc.tensor.matmul(
                            psum[:, h * 512 : (h + 1) * 512],
                            lhsT=f_aug[:, kt * 128 : (kt + 1) * 128],
                            rhs=x_aug[:, mh * 1024 + h * 512 :
                                      mh * 1024 + (h + 1) * 512],
                            start=True, stop=True,
                        )
                    st = stages.tile([128, 1024], BF16, tag="stage")
                    if z_acc is None:
                        nc.scalar.activation(st, psum, AF.Exp)
                    else:
                        zt = zpool.tile([128, 1], F32, tag="zt")
                        nc.scalar.activation(st, psum, AF.Exp, accum_out=zt)
                        nc.vector.tensor_add(
                            z_acc[:, kt : kt + 1], z_acc[:, kt : kt + 1], zt
                        )
                    nc.sync.dma_start(
                        dst[mh * 4 : (mh + 1) * 4, :, kt, :].rearrange(
                            "c kp m -> kp c m"),
                        st.rearrange("kp (c m) -> kp c m", m=256),
                    )
                    yield

        for _ in gen_units(f_aug_g, xt, P_d, None):
            pass

    if _stop_here("ph1"):
        return

    # =============== chain stages ===============
    agen = gen_units(f_aug_e, xe_aug, A_d, Z)

    with tc.tile_pool(name="strips", bufs=2) as strips:

        def chain_stage(src_dram, rhs_T, out_cb, post_group=None):
            for mg in range(NT // 2):
                strip = strips.tile([128, NT, 256], BF16, tag="strip")
                nc.sync.dma_start(strip, src_dram[mg])
                for mi in range(2):
                    mt = mg * 2 + mi
                    psum = pc.tile([128, 512], F32, tag="pc")
                    for kt in range(NT):
                        nc.tensor.matmul(
                            psum,
                            lhsT=strip[:, kt, mi * 128 : (mi + 1) * 128],
                            rhs=rhs_T[:, kt, :],
                            start=(kt == 0),
                            stop=(kt == NT - 1),
                        )
                    out_cb(mt, psum)
                if post_group is not None:
                    post_group()

        def drive(gen, n):
            def _f():
                for _ in range(n):
                    try:
                        next(gen)
                    except StopIteration:
                        return
            return _f

        def copy_to(dstT):
            def _cb(mt, psum):
                nc.vector.tensor_copy(dstT[:, mt, :], psum)
            return _cb

        # g1: T0g -> TA, interleave A generation
        TA = chain.tile([128, NT, 512], BF16, tag="T")
        chain_stage(P_d, T0g, copy_to(TA), post_group=drive(agen, 8))

        if _stop_here("g1"):
            return

        # Z -> Zinv, -logZ
        Zinv = pers.tile([128, NT], F32, tag="Zinv")
        nc.vector.reciprocal(Zinv, Z)
        nlz = pers.tile([128, NT], F32, tag="nlz")
        nc.scalar.activation(nlz, Z, AF.Ln)
        nc.vector.tensor_scalar(nlz, nlz, -1.0, -C0, OP.mult, OP.add)

        if _stop_here("g2a"):
            return

        # T0e units (interleaved into g2): exp(logits - logZr_i - logZ_j)
        def t0e_units():
            for jt in range(NT):
                psum = pg.tile([128, 1024], F32, tag="pg")
                nc.tensor.matmul(
                    psum[:, :512],
                    lhsT=xe_aug[:, jt * 128 : (jt + 1) * 128],
                    rhs=f_aug_er2,
                    start=True, stop=True,
                )
                nc.scalar.activation(
                    T0e[:, jt, :], psum[:, :512], AF.Exp,
                    bias=nlz[:, jt : jt + 1],
                )
                yield

        t0gen = t0e_units()
        if stop in ("g3b", "e1b", "g3c", "g3d", "g3e", "g3f"):
            for _ in t0gen:
                pass
        if stop == "g2b":
            for _ in t0gen:
                pass
            if _stop_here("g2b"):
                return
        # g2: TA -> TB, interleave T0e generation
        TB = chain.tile([128, NT, 512], BF16, tag="T")
        if stop == "g2c":
            chain_stage(P_d, TA, copy_to(TB))
            if _stop_here("g2c"):
                return
        chain_stage(P_d, TA, copy_to(TB), post_group=drive(t0gen, 2))

        if _stop_here("g2"):
            return

        # g3: TB -> (loss term1, T3g bf16 kept)
        T3g = chain.tile([128, NT, 512], BF16, tag="T")

        if stop in ("g3c", "g3d", "g3e", "g3f"):
            # g3c = plain copy; g3d = +Ln; g3e = +ACT-copy out; g3f = +ttr->fresh
            def g3v_cb(mt, psum):
                if stop in ("g3d", "g3e", "g3f"):
                    ltmp = lpool.tile([128, 512], F32, tag="ltmp")
                    nc.scalar.activation(ltmp, psum, AF.Ln)
                if stop == "g3f":
                    ptmp = lpool.tile([128, 512], F32, tag="ptmp")
                    racc_t = zpool.tile([128, 1], F32, tag="zt")
                    nc.vector.tensor_tensor_reduce(
                        out=ptmp, in0=psum, in1=ltmp, scale=1.0,
                        scalar=0.0, op0=OP.mult, op1=OP.add,
                        accum_out=racc_t)
                    nc.vector.tensor_add(acc1, acc1, racc_t)
                if stop == "g3e":
                    nc.scalar.activation(T3g[:, mt, :], psum, AF.Copy)
                else:
                    nc.vector.tensor_copy(T3g[:, mt, :], psum)
            chain_stage(P_d, TB, g3v_cb)
            if _stop_here(stop):
                return

        def g3_cb(mt, psum):
            ltmp = lpool.tile([128, 512], F32, tag="ltmp")
            nc.scalar.activation(ltmp, psum, AF.Ln)
            ptmp = lpool.tile([128, 512], F32, tag="ptmp")
            racc_t = zpool.tile([128, 1], F32, tag="zt")
            nc.vector.tensor_mul(ptmp, psum, ltmp)
            nc.vector.tensor_reduce(racc_t, ptmp, mybir.AxisListType.X, OP.add)
            nc.vector.tensor_add(acc1, acc1, racc_t)
            nc.scalar.activation(T3g[:, mt, :], psum, AF.Copy)
            if mt == 0 and stop == "g3b":
                nc.sync.dma_start(OUT["dbgp"].ap(), ptmp)

        chain_stage(P_d, TB, g3_cb)

        if _stop_here("g3"):
            return

        # e1: T0e -> TE1 (scaled by 1/Z), e2: TE1 -> TE2
        def scaled_copy_to(dstT):
            def _cb(mt, psum):
                nc.vector.tensor_scalar_mul(
                    dstT[:, mt, :], psum, Zinv[:, mt : mt + 1]
                )
            return _cb

        if stop == "g3b":
            nc.sync.dma_start(OUT["dbg0"].ap(), T0g[:, 0, :])
            nc.sync.dma_start(OUT["dbg1"].ap(), TA[:, 0, :])
            nc.sync.dma_start(OUT["dbg2"].ap(), TB[:, 0, :])
            if _stop_here("g3b"):
                return

        TE1 = chain.tile([128, NT, 512], BF16, tag="T")
        chain_stage(A_d, T0e, scaled_copy_to(TE1))
        if stop == "e1b" and _stop_here("e1b"):
            return
        TE2 = chain.tile([128, NT, 512], BF16, tag="T")
        chain_stage(A_d, TE1, scaled_copy_to(TE2))

        if _stop_here("e2"):
            return

        # e3: loss term2 = sum(T3g * log T3e)
        def e3_cb(mt, psum):
            ltmp = lpool.tile([128, 512], F32, tag="ltmp")
            nc.scalar.activation(ltmp, psum, AF.Ln)
            ptmp = lpool.tile([128, 512], F32, tag="ptmp")
            racc_t = zpool.tile([128, 1], F32, tag="zt")
            nc.vector.tensor_mul(ptmp, ltmp, T3g[:, mt, :])
            nc.vector.tensor_reduce(racc_t, ptmp, mybir.AxisListType.X, OP.add)
            nc.vector.tensor_add(acc2, acc2, racc_t)

        chain_stage(A_d, TE2, e3_cb)

    nc.sync.dma_start(OUT["acc1_out"].ap(), acc1)
    nc.sync.dma_start(OUT["acc2_out"].ap(), acc2)


_CACHE = {}


def build_program(stop="full"):
    if stop in _CACHE:
        return _CACHE[stop]
    nc = bacc.Bacc("TRN2", target_bir_lowering=False, debug=False,
                   num_devices=NCORES)
    IN = {}
    shapes = [
        ("xT_aug", [D + 1, N], BF16), ("flowsT", [D, N], BF16),
        ("xrT_aug", [D + 1, R], BF16), ("flowsrT", [D, R], BF16),
    ]
    for name, (k, m) in WSHAPES:
        shapes.append((name, [k, m], BF16))
        shapes.append((name + "b", [m, 1], F32))
    for name, shape, dt in shapes:
        IN[name] = nc.dram_tensor(name, shape, dt, kind="ExternalInput")
    OUT = {}
    for name, shape in [
        ("acc1_out", [128, 1]), ("acc2_out", [128, 1]), ("recon_out", [D, 1]),
    ]:
        OUT[name] = nc.dram_tensor(name, shape, F32, kind="ExternalOutput")
    for name in ["dbg0", "dbg1", "dbg2"]:
        OUT[name] = nc.dram_tensor(name, [128, 512], BF16, kind="ExternalOutput")
    OUT["dbgp"] = nc.dram_tensor("dbgp", [128, 512], F32, kind="ExternalOutput")

    from contextlib import ExitStack

    with tile.TileContext(nc) as tc, ExitStack() as ctx:
        _emit(ctx, tc, IN, OUT, stop=stop)
    nc.compile()
    _CACHE[stop] = nc
    return nc


def make_in_maps(X, flows, enc_params, dec_params, fa_params):
    bf = ml_dtypes.bfloat16
    X = np.asarray(X, np.float32)
    flows = np.asarray(flows, np.float32)

    xT_aug = np.ascontiguousarray(
        np.concatenate([X.T, np.ones((1, N), np.float32)], axis=0).astype(bf))
    # pre-scaled by 1/sigma: the device uses this directly as f_aug rows
    flowsT = np.ascontiguousarray((2.0 * flows.T).astype(bf))
    common = {"xT_aug": xT_aug, "flowsT": flowsT}
    for prefix, params in [("enc", enc_params), ("dec", dec_params),
                           ("fa", fa_params)]:
        for i in range(3):
            Wm = np.asarray(params[2 * i], np.float32)
            bv = np.asarray(params[2 * i + 1], np.float32)
            if prefix == "fa" and i == 2:
                Wm = Wm * 2.0            # fa3 output feeds f_aug_e (2*ef^T)
                bv = bv * 2.0
            common[f"{prefix}{i + 1}"] = np.ascontiguousarray(Wm.astype(bf))
            common[f"{prefix}{i + 1}b"] = np.ascontiguousarray(
                bv[:, None].astype(np.float32))
    in_maps = []
    for c in range(NCORES):
        rows = slice(c * R, (c + 1) * R)
        m = dict(common)
        m["xrT_aug"] = np.ascontiguousarray(xT_aug[:, rows])
        m["flowsrT"] = np.ascontiguousarray(flowsT[:, rows])
        in_maps.append(m)
    return in_maps


def combine(results):
    t1 = np.float64(0.0)
    t2 = np.float64(0.0)
    rc = np.float64(0.0)
    for r in results:
        t1 += np.sum(r["acc1_out"].astype(np.float64))
        t2 += np.sum(r["acc2_out"].astype(np.float64))
        rc += np.sum(r["recon_out"].astype(np.float64))
    loss = (t1 - t2) / N + rc / (N * D)
    return np.float32(loss)


def kernel(X, flows, enc_params, dec_params, fa_params):
    nc = build_program()
    in_maps = make_in_maps(X, flows, enc_params, dec_params, fa_params)
    res = run_bass_kernel_spmd(nc, in_maps, core_ids=list(range(NCORES)))
    return combine(res.results)


# revision 27
# speedup vs baseline: 1.0156x; 1.0156x over previous
"""
DiffusionFlowEmbedder loss kernel for 8 Trainium2 NeuronCores.

Math (n=4096, d=100, e=2):
  P = exp((flows @ X.T - c'_i)/sigma),  c'_i = ||f_i||^2 + x_i.f_i   (n x n)
  loss = sum(P^4 * (log P^4 - log Pemb^4))/n + mean((recon - X)^2)
  where Pemb = rownorm(exp affinity of the encoder embedding).

Strategy: row-shard the n x n work over 8 cores with ZERO collectives.
Each core owns 512 rows and computes T_s = (M^s [rows,:])^T via the
recurrence T_{s+1} = matmul(lhsT=M, rhs=T_s) (out = M^T @ T_s), where the
full M (P_graph or A_emb) is generated on-device once (affinity matmul +
exp on the scalar engine) into DRAM bf16 and streamed back as the
stationary operand. Affinity row constants (-2c') ride the contraction
as an augmentation row; row-normalization of A is folded into exp biases
(-log Z) and per-partition PSUM->SBUF copy scales (1/Z), with Z obtained
free via the activation accum_out. The per-core output is three partial
sum vectors; the host reduces them to the final scalar.
"""

import numpy as np
import ml_dtypes

import concourse.bass as bass
import concourse.mybir as mybir
import concourse.tile as tile
from concourse import bacc
from concourse.bass_utils import run_bass_kernel_spmd

F32 = mybir.dt.float32
BF16 = mybir.dt.bfloat16
AF = mybir.ActivationFunctionType
OP = mybir.AluOpType

NCORES = 8
N = 4096
D = 100
E = 2
R = N // NCORES          # 512 rows per core
NT = N // 128            # 32 partition tiles
RT = R // 128            # 4
ISIG = 2.0               # 1/sigma
C0 = 8.3125              # ~log(4096), exact in bf16; centers the -logZr row

WSHAPES = [
    ("enc1", [D, D]), ("enc2", [D, 10]), ("enc3", [10, E]),
    ("fa1", [E, 10]), ("fa2", [10, 10]), ("fa3", [10, E]),
    ("dec1", [E, 10]), ("dec2", [10, D]), ("dec3", [D, D]),
]


def _emit(ctx, tc, IN, OUT, stop="full"):
    nc = tc.nc

    def _stop_here(name):
        if stop != name:
            return False
        nc.sync.dma_start(OUT["acc1_out"].ap(), acc1)
        nc.sync.dma_start(OUT["acc2_out"].ap(), acc2)
        return True

    pers = ctx.enter_context(tc.tile_pool(name="pers", bufs=1))
    stages = ctx.enter_context(tc.tile_pool(name="stages", bufs=3))
    lpool = ctx.enter_context(tc.tile_pool(name="lpool", bufs=2))
    zpool = ctx.enter_context(tc.tile_pool(name="zpool", bufs=4))
    crows = ctx.enter_context(tc.tile_pool(name="crows", bufs=2))
    chain = ctx.enter_context(tc.tile_pool(name="chain", bufs=3))
    dram = ctx.enter_context(tc.tile_pool(name="dram", bufs=1, space="DRAM"))
    pc = ctx.enter_context(tc.tile_pool(name="pc", bufs=3, space="PSUM"))
    pg = ctx.enter_context(tc.tile_pool(name="pg", bufs=2, space="PSUM"))
    ps = ctx.enter_context(tc.tile_pool(name="ps", bufs=1, space="PSUM"))

    # strip-major layout: [m-group, partition, k-tile, m-within-group] so a
    # chain strip is one fully-contiguous-per-partition DMA read.
    P_d = dram.tile([NT // 2, 128, NT, 256], BF16, tag="P_d")
    A_d = dram.tile([NT // 2, 128, NT, 256], BF16, tag="A_d")

    def sbin(pool, name, shape, dtype=BF16):
        t = pool.tile(shape, dtype, tag=name)
        nc.sync.dma_start(t, IN[name].ap())
        return t

    # persistent small inputs
    xr = sbin(pers, "xrT_aug", [D + 1, R])
    fr = sbin(pers, "flowsrT", [D, R])       # pre-scaled 2*flows^T rows
    W = {}
    B = {}
    for name, (k, m) in WSHAPES:
        W[name] = sbin(pers, name, [k, m])
        B[name] = sbin(pers, name + "b", [m, 1], F32)

    ones = pers.tile([128, 1], BF16, tag="ones")
    nc.vector.memset(ones, 1.0)
    onesrow = pers.tile([1, N], BF16, tag="onesrow")
    nc.vector.memset(onesrow, 1.0)
    acc1 = pers.tile([128, 1], F32, tag="acc1")
    acc2 = pers.tile([128, 1], F32, tag="acc2")
    nc.vector.memset(acc1, 0.0)
    nc.vector.memset(acc2, 0.0)

    def linear(name, rhs, out, func):
        w = W[name]
        m = w.shape[-1]
        cols = rhs.shape[-1]
        psum = ps.tile([128, 512], F32, tag="ps")
        nc.tensor.matmul(psum[:m, :cols], lhsT=w, rhs=rhs, start=True, stop=True)
        nc.scalar.activation(out, psum[:m, :cols], func, bias=B[name])

    # long-lived mid tiles
    xe_aug = pers.tile([E + 1, N], BF16, tag="xe_aug")   # [emb^T; 1]
    nc.sync.dma_start(xe_aug[E : E + 1, :], onesrow)
    f_aug_e = pers.tile([E + 1, N], BF16, tag="f_aug_e")
    Z = pers.tile([128, NT], F32, tag="Z")
    nc.vector.memset(Z, 0.0)
    T0g = chain.tile([128, NT, 512], BF16, tag="T")
    T0e = chain.tile([128, NT, 512], BF16, tag="T")

    # =============== phase 1: prep (scoped tiles) ===============
    # host feeds flowsT/flowsrT pre-scaled by 2 (= 1/sigma): the input IS
    # the f_aug main block. c' sums then carry a 4x factor folded into the
    # -0.5 copy scale: (2x + 2f) . 2f = 4(x.f + f.f),  -2c' = -0.5 * that.
    with tc.tile_pool(name="ph1", bufs=1) as ph1:
        xt = sbin(ph1, "xT_aug", [D + 1, N])
        f_aug_g = ph1.tile([D + 1, N], BF16, tag="f_aug_g")
        nc.sync.dma_start(f_aug_g[:D, :], IN["flowsT"].ap())

        # ---- affinity aug rows: -2c' staged at partition 0, DMA'd in ----
        sg = ph1.tile([D, N], BF16, tag="sgse")
        nc.vector.scalar_tensor_tensor(
            sg, in0=xt[:D, :], scalar=ISIG, in1=f_aug_g[:D, :],
            op0=OP.mult, op1=OP.add)
        nc.vector.tensor_mul(sg, sg, f_aug_g[:D, :])
        for sl in range(N // 512):
            s = slice(sl * 512, (sl + 1) * 512)
            psum = ps.tile([128, 512], F32, tag="ps")
            nc.tensor.matmul(psum[:1, :], lhsT=ones[:D, :], rhs=sg[:, s],
                             start=True, stop=True)
            cr = crows.tile([1, 512], BF16, tag="crow")
            nc.scalar.activation(cr, psum[:1, :], AF.Copy, scale=-0.5)
            nc.sync.dma_start(f_aug_g[D : D + 1, s], cr)

        # f_aug_gr / f_aug_er aug rows (-2c'); fr is pre-scaled 2*flows^T
        f_aug_gr = pers.tile([D + 1, R], BF16, tag="f_aug_gr")
        nc.vector.tensor_copy(f_aug_gr[:D, :], fr)
        sgr = pers.tile([D, R], BF16, tag="sgr")
        nc.vector.scalar_tensor_tensor(
            sgr, in0=xr[:D, :], scalar=ISIG, in1=fr, op0=OP.mult, op1=OP.add)
        nc.vector.tensor_mul(sgr, sgr, fr)
        psum = ps.tile([128, 512], F32, tag="ps")
        nc.tensor.matmul(psum[:1, :R], lhsT=ones[:D, :], rhs=sgr, start=True,
                         stop=True)
        cgr = crows.tile([1, 512], BF16, tag="crow")
        nc.scalar.activation(cgr, psum[:1, :R], AF.Copy, scale=-0.5)
        nc.sync.dma_start(f_aug_gr[D : D + 1, :], cgr)

        # ---- T0g: (P[rows,:])^T  [4096, 512] ----
        for jt in range(NT):
            psum = pg.tile([128, 1024], F32, tag="pg")
            nc.tensor.matmul(
                psum[:, :512],
                lhsT=xt[:, jt * 128 : (jt + 1) * 128],
                rhs=f_aug_gr,
                start=True, stop=True,
            )
            nc.scalar.activation(T0g[:, jt, :], psum[:, :512], AF.Exp)

        # ---- P generation (standalone; nothing else ready yet) ----
        def gen_units(f_aug, x_aug, dst, z_acc):
            for mh in range(4):
                for kt in range(NT):
                    psum = pg.tile([128, 1024], F32, tag="pg")
                    for h in range(2):
                        nc.tensor.matmul(
                            psum[:, h * 512 : (h + 1) * 512],
                            lhsT=f_aug[:, kt * 128 : (kt + 1) * 128],
                            rhs=x_aug[:, mh * 1024 + h * 512 :
                                      mh * 1024 + (h + 1) * 512],
                            start=True, stop=True,
                        )
                    st = stages.tile([128, 1024], BF16, tag="stage")
                    if z_acc is None:
                        nc.scalar.activation(st, psum, AF.Exp)
                    else:
                        zt = zpool.tile([128, 1], F32, tag="zt")
                        nc.scalar.activation(st, psum, AF.Exp, accum_out=zt)
                        nc.vector.tensor_add(
                            z_acc[:, kt : kt + 1], z_acc[:, kt : kt + 1], zt
                        )
                    nc.sync.dma_start(
                        dst[mh * 4 : (mh + 1) * 4, :, kt, :].rearrange(
                            "c kp m -> kp c m"),
                        st.rearrange("kp (c m) -> kp c m", m=256),
                    )
                    yield

        for _ in gen_units(f_aug_g, xt, P_d, None):
            pass

        # ---- full-width MLPs (transposed layout, bias via ACT) ----
        h1a = ph1.tile([D, N], BF16, tag="tmpA")
        h2a = ph1.tile([10, N], BF16, tag="tmpB")
        for sl in range(N // 512):
            s = slice(sl * 512, (sl + 1) * 512)
            linear("enc1", xt[:D, s], h1a[:, s], AF.Relu)
            linear("enc2", h1a[:, s], h2a[:, s], AF.Relu)
            linear("enc3", h2a[:, s], xe_aug[:E, s], AF.Identity)
        g1a = ph1.tile([10, N], BF16, tag="tmpA")
        g2a = ph1.tile([10, N], BF16, tag="tmpB")
        for sl in range(N // 512):
            s = slice(sl * 512, (sl + 1) * 512)
            linear("fa1", xe_aug[:E, s], g1a[:, s], AF.Tanh)
            linear("fa2", g1a[:, s], g2a[:, s], AF.Tanh)
            # ef^T * 2 straight into f_aug_e rows (fa3 W,b pre-scaled by 2)
            linear("fa3", g2a[:, s], f_aug_e[:E, s], AF.Identity)

        se = ph1.tile([E, N], BF16, tag="sgse")
        nc.vector.scalar_tensor_tensor(
            se, in0=xe_aug[:E, :], scalar=ISIG, in1=f_aug_e[:E, :],
            op0=OP.mult, op1=OP.add)
        nc.vector.tensor_mul(se, se, f_aug_e[:E, :])
        for sl in range(N // 512):
            s = slice(sl * 512, (sl + 1) * 512)
            psum = ps.tile([128, 512], F32, tag="ps")
            nc.tensor.matmul(psum[:1, :], lhsT=ones[:E, :], rhs=se[:, s],
                             start=True, stop=True)
            cr = crows.tile([1, 512], BF16, tag="crow")
            nc.scalar.activation(cr, psum[:1, :], AF.Copy, scale=-0.5)
            nc.sync.dma_start(f_aug_e[E : E + 1, s], cr)

        # ---- per-core rows versions ----
        xer = pers.tile([E + 1, R], BF16, tag="xer")
        nc.sync.dma_start(xer[E : E + 1, :], onesrow[0:1, :R])
        h1r = pers.tile([D, R], BF16, tag="h1r")
        h2r = pers.tile([10, R], BF16, tag="h2r")
        f_aug_er = pers.tile([E + 1, R], BF16, tag="f_aug_er")
        linear("enc1", xr[:D, :], h1r, AF.Relu)
        linear("enc2", h1r, h2r, AF.Relu)
        linear("enc3", h2r, xer[:E, :], AF.Identity)
        linear("fa1", xer[:E, :], h2r, AF.Tanh)
        linear("fa2", h2r, h1r[:10, :], AF.Tanh)
        linear("fa3", h1r[:10, :], f_aug_er[:E, :], AF.Identity)

        # recon loss partial: dec MLP on emb rows
        linear("dec1", xer[:E, :], h2r, AF.Relu)
        linear("dec2", h2r, h1r, AF.Relu)
        psum = ps.tile([128, 512], F32, tag="ps")
        nc.tensor.matmul(psum[:D, :R], lhsT=W["dec3"], rhs=h1r, start=True,
                         stop=True)
        rec = lpool.tile([128, 512], F32, tag="ltmp")
        nc.scalar.activation(rec[:D, :R], psum[:D, :R], AF.Identity,
                             bias=B["dec3"])
        diff = lpool.tile([128, 512], F32, tag="ptmp")
        nc.vector.tensor_sub(diff[:D, :R], rec[:D, :R], xr[:D, :])
        sq = lpool.tile([128, 512], F32, tag="ltmp")
        racc = pers.tile([D, 1], F32, tag="racc")
        nc.scalar.activation(sq[:D, :R], diff[:D, :R], AF.Square,
                             accum_out=racc)
        nc.sync.dma_start(OUT["recon_out"].ap(), racc)

        ser = pers.tile([E, R], BF16, tag="ser")
        nc.vector.scalar_tensor_tensor(
            ser, in0=xer[:E, :], scalar=ISIG, in1=f_aug_er[:E, :],
            op0=OP.mult, op1=OP.add)
        nc.vector.tensor_mul(ser, ser, f_aug_er[:E, :])
        psum = ps.tile([128, 512], F32, tag="ps")
        nc.tensor.matmul(psum[:1, :R], lhsT=ones[:E, :], rhs=ser, start=True,
                         stop=True)
        cer = pers.tile([1, R], BF16, tag="cer")   # keep: reused for er2
        nc.scalar.activation(cer, psum[:1, :R], AF.Copy, scale=-0.5)
        nc.sync.dma_start(f_aug_er[E : E + 1, :], cer)

        # ---- Z_rows pass: row sums of A[rows, :] ----
        Zr = pers.tile([128, RT], F32, tag="Zr")
        nc.vector.memset(Zr, 0.0)
        junk = pers.tile([128, 512], BF16, tag="junk")
        for it in range(RT):
            for jg in range(N // 512):
                psum = pg.tile([128, 1024], F32, tag="pg")
                nc.tensor.matmul(
                    psum[:, :512],
                    lhsT=f_aug_er[:, it * 128 : (it + 1) * 128],
                    rhs=xe_aug[:, jg * 512 : (jg + 1) * 512],
                    start=True, stop=True,
                )
                zt = zpool.tile([128, 1], F32, tag="zt")
                nc.scalar.activation(junk, psum[:, :512], AF.Exp, accum_out=zt)
                nc.vector.tensor_add(Zr[:, it : it + 1], Zr[:, it : it + 1], zt)

        # -log(Zr) shuffled from [128, RT] (partition-major) to a [1, R] row
        lzr = pers.tile([128, RT], F32, tag="lzr")
        nc.scalar.activation(lzr, Zr, AF.Ln)
        nc.vector.tensor_scalar_mul(lzr, lzr, -1.0)
        zrow = pers.tile([1, R], F32, tag="zrow")
        ztmp_d = dram.tile([128, RT], F32, tag="ztmp_d")
        nc.sync.dma_start(ztmp_d, lzr)
        with nc.allow_non_contiguous_dma(reason="512-elem partition->row shuffle"):
            nc.sync.dma_start(
                zrow.rearrange("one (t p) -> one t p", p=128),
                ztmp_d.rearrange("p t -> t p")[None],
            )
        # center the aug row near 0 (logZr ~ log(4096) ~ 8.318 would quantize
        # to bf16 step 0.0625 -> systematic T0e shift); C0 is re-added via the
        # fp32 ACT bias of the T0e exp below.
        f_aug_er2 = pers.tile([E + 1, R], BF16, tag="f_aug_er2")
        nc.vector.tensor_copy(f_aug_er2[:E, :], f_aug_er[:E, :])
        zrow2 = pers.tile([1, R], BF16, tag="zrow2")
        nc.vector.scalar_tensor_tensor(
            zrow2, in0=zrow, scalar=C0, in1=cer, op0=OP.add, op1=OP.add)
        nc.sync.dma_start(f_aug_er2[E : E + 1, :], zrow2)


    if _stop_here("ph1"):
        return

    # =============== chain stages ===============
    agen = gen_units(f_aug_e, xe_aug, A_d, Z)

    with tc.tile_pool(name="strips", bufs=2) as strips:

        def chain_stage(src_dram, rhs_T, out_cb, post_group=None):
            for mg in range(NT // 2):
                strip = strips.tile([128, NT, 256], BF16, tag="strip")
                nc.sync.dma_start(strip, src_dram[mg])
                for mi in range(2):
                    mt = mg * 2 + mi
                    psum = pc.tile([128, 512], F32, tag="pc")
                    for kt in range(NT):
                        nc.tensor.matmul(
                            psum,
                            lhsT=strip[:, kt, mi * 128 : (mi + 1) * 128],
                            rhs=rhs_T[:, kt, :],
                            start=(kt == 0),
                            stop=(kt == NT - 1),
                        )
                    out_cb(mt, psum)
                if post_group is not None:
                    post_group()

        def drive(gen, n):
            def _f():
                for _ in range(n):
                    try:
                        next(gen)
                    except StopIteration:
                        return
            return _f

        def copy_to(dstT):
            def _cb(mt, psum):
                nc.vector.tensor_copy(dstT[:, mt, :], psum)
            return _cb

        # g1: T0g -> TA, interleave A generation
        TA = chain.tile([128, NT, 512], BF16, tag="T")
        chain_stage(P_d, T0g, copy_to(TA), post_group=drive(agen, 8))

        if _stop_here("g1"):
            return

        # Z -> Zinv, -logZ
        Zinv = pers.tile([128, NT], F32, tag="Zinv")
        nc.vector.reciprocal(Zinv, Z)
        nlz = pers.tile([128, NT], F32, tag="nlz")
        nc.scalar.activation(nlz, Z, AF.Ln)
        nc.vector.tensor_scalar(nlz, nlz, -1.0, -C0, OP.mult, OP.add)

        if _stop_here("g2a"):
            return

        # T0e units (interleaved into g2): exp(logits - logZr_i - logZ_j)
        def t0e_units():
            for jt in range(NT):
                psum = pg.tile([128, 1024], F32, tag="pg")
                nc.tensor.matmul(
                    psum[:, :512],
                    lhsT=xe_aug[:, jt * 128 : (jt + 1) * 128],
                    rhs=f_aug_er2,
                    start=True, stop=True,
                )
                nc.scalar.activation(
                    T0e[:, jt, :], psum[:, :512], AF.Exp,
                    bias=nlz[:, jt : jt + 1],
                )
                yield

        t0gen = t0e_units()
        if stop in ("g3b", "e1b", "g3c", "g3d", "g3e", "g3f"):
            for _ in t0gen:
                pass
        if stop == "g2b":
            for _ in t0gen:
                pass
            if _stop_here("g2b"):
                return
        # g2: TA -> TB, interleave T0e generation
        TB = chain.tile([128, NT, 512], BF16, tag="T")
        if stop == "g2c":
            chain_stage(P_d, TA, copy_to(TB))
            if _stop_here("g2c"):
                return
        chain_stage(P_d, TA, copy_to(TB), post_group=drive(t0gen, 2))

        if _stop_here("g2"):
            return

        # g3: TB -> (loss term1, T3g bf16 kept)
        T3g = chain.tile([128, NT, 512], BF16, tag="T")

        if stop in ("g3c", "g3d", "g3e", "g3f"):
            # g3c = plain copy; g3d = +Ln; g3e = +ACT-copy out; g3f = +ttr->fresh
            def g3v_cb(mt, psum):
                if stop in ("g3d", "g3e", "g3f"):
                    ltmp = lpool.tile([128, 512], F32, tag="ltmp")
                    nc.scalar.activation(ltmp, psum, AF.Ln)
                if stop == "g3f":
                    ptmp = lpool.tile([128, 512], F32, tag="ptmp")
                    racc_t = zpool.tile([128, 1], F32, tag="zt")
                    nc.vector.tensor_tensor_reduce(
                        out=ptmp, in0=psum, in1=ltmp, scale=1.0,
                        scalar=0.0, op0=OP.mult, op1=OP.add,
                        accum_out=racc_t)
                    nc.vector.tensor_add(acc1, acc1, racc_t)
                if stop == "g3e":
                    nc.scalar.activation(T3g[:, mt, :], psum, AF.Copy)
                else:
                    nc.vector.tensor_copy(T3g[:, mt, :], psum)
            chain_stage(P_d, TB, g3v_cb)
            if _stop_here(stop):
                return

        def g3_cb(mt, psum):
            ltmp = lpool.tile([128, 512], F32, tag="ltmp")
            nc.scalar.activation(ltmp, psum, AF.Ln)
            ptmp = lpool.tile([128, 512], F32, tag="ptmp")
            racc_t = zpool.tile([128, 1], F32, tag="zt")
            nc.vector.tensor_mul(ptmp, psum, ltmp)
            nc.vector.tensor_reduce(racc_t, ptmp, mybir.AxisListType.X, OP.add)
            nc.vector.tensor_add(acc1, acc1, racc_t)
            nc.scalar.activation(T3g[:, mt, :], psum, AF.Copy)
            if mt == 0 and stop == "g3b":
                nc.sync.dma_start(OUT["dbgp"].ap(), ptmp)

        chain_stage(P_d, TB, g3_cb)

        if _stop_here("g3"):
            return

        # e1: T0e -> TE1 (scaled by 1/Z), e2: TE1 -> TE2
        def scaled_copy_to(dstT):
            def _cb(mt, psum):
                nc.vector.tensor_scalar_mul(
                    dstT[:, mt, :], psum, Zinv[:, mt : mt + 1]
                )
            return _cb

        if stop == "g3b":
            nc.sync.dma_start(OUT["dbg0"].ap(), T0g[:, 0, :])
            nc.sync.dma_start(OUT["dbg1"].ap(), TA[:, 0, :])
            nc.sync.dma_start(OUT["dbg2"].ap(), TB[:, 0, :])
            if _stop_here("g3b"):
                return

        TE1 = chain.tile([128, NT, 512], BF16, tag="T")
        chain_stage(A_d, T0e, scaled_copy_to(TE1))
        if stop == "e1b" and _stop_here("e1b"):
            return
        TE2 = chain.tile([128, NT, 512], BF16, tag="T")
        chain_stage(A_d, TE1, scaled_copy_to(TE2))

        if _stop_here("e2"):
            return

        # e3: loss term2 = sum(T3g * log T3e)
        def e3_cb(mt, psum):
            ltmp = lpool.tile([128, 512], F32, tag="ltmp")
            nc.scalar.activation(ltmp, psum, AF.Ln)
            ptmp = lpool.tile([128, 512], F32, tag="ptmp")
            racc_t = zpool.tile([128, 1], F32, tag="zt")
            nc.vector.tensor_mul(ptmp, ltmp, T3g[:, mt, :])
            nc.vector.tensor_reduce(racc_t, ptmp, mybir.AxisListType.X, OP.add)
            nc.vector.tensor_add(acc2, acc2, racc_t)

        chain_stage(A_d, TE2, e3_cb)

    nc.sync.dma_start(OUT["acc1_out"].ap(), acc1)
    nc.sync.dma_start(OUT["acc2_out"].ap(), acc2)


_CACHE = {}


def build_program(stop="full"):
    if stop in _CACHE:
        return _CACHE[stop]
    nc = bacc.Bacc("TRN2", target_bir_lowering=False, debug=False,
                   num_devices=NCORES)
    IN = {}
    shapes = [
        ("xT_aug", [D + 1, N], BF16), ("flowsT", [D, N], BF16),
        ("xrT_aug", [D + 1, R], BF16), ("flowsrT", [D, R], BF16),
    ]
    for name, (k, m) in WSHAPES:
        shapes.append((name, [k, m], BF16))
        shapes.append((name + "b", [m, 1], F32))
    for name, shape, dt in shapes:
        IN[name] = nc.dram_tensor(name, shape, dt, kind="ExternalInput")
    OUT = {}
    for name, shape in [
        ("acc1_out", [128, 1]), ("acc2_out", [128, 1]), ("recon_out", [D, 1]),
    ]:
        OUT[name] = nc.dram_tensor(name, shape, F32, kind="ExternalOutput")
    for name in ["dbg0", "dbg1", "dbg2"]:
        OUT[name] = nc.dram_tensor(name, [128, 512], BF16, kind="ExternalOutput")
    OUT["dbgp"] = nc.dram_tensor("dbgp", [128, 512], F32, kind="ExternalOutput")

    from contextlib import ExitStack

    with tile.TileContext(nc) as tc, ExitStack() as ctx:
        _emit(ctx, tc, IN, OUT, stop=stop)
    nc.compile()
    _CACHE[stop] = nc
    return nc


def make_in_maps(X, flows, enc_params, dec_params, fa_params):
    bf = ml_dtypes.bfloat16
    X = np.asarray(X, np.float32)
    flows = np.asarray(flows, np.float32)

    xT_aug = np.ascontiguousarray(
        np.concatenate([X.T, np.ones((1, N), np.float32)], axis=0).astype(bf))
    # pre-scaled by 1/sigma: the device uses this directly as f_aug rows
    flowsT = np.ascontiguousarray((2.0 * flows.T).astype(bf))
    common = {"xT_aug": xT_aug, "flowsT": flowsT}
    for prefix, params in [("enc", enc_params), ("dec", dec_params),
                           ("fa", fa_params)]:
        for i in range(3):
            Wm = np.asarray(params[2 * i], np.float32)
            bv = np.asarray(params[2 * i + 1], np.float32)
            if prefix == "fa" and i == 2:
                Wm = Wm * 2.0            # fa3 output feeds f_aug_e (2*ef^T)
                bv = bv * 2.0
            common[f"{prefix}{i + 1}"] = np.ascontiguousarray(Wm.astype(bf))
            common[f"{prefix}{i + 1}b"] = np.ascontiguousarray(
                bv[:, None].astype(np.float32))
    in_maps = []
    for c in range(NCORES):
        rows = slice(c * R, (c + 1) * R)
        m = dict(common)
        m["xrT_aug"] = np.ascontiguousarray(xT_aug[:, rows])
        m["flowsrT"] = np.ascontiguousarray(flowsT[:, rows])
        in_maps.append(m)
    return in_maps


def combine(results):
    t1 = np.float64(0.0)
    t2 = np.float64(0.0)
    rc = np.float64(0.0)
    for r in results:
        t1 += np.sum(r["acc1_out"].astype(np.float64))
        t2 += np.sum(r["acc2_out"].astype(np.float64))
        rc += np.sum(r["recon_out"].astype(np.float64))
    loss = (t1 - t2) / N + rc / (N * D)
    return np.float32(loss)


def kernel(X, flows, enc_params, dec_params, fa_params):
    nc = build_program()
    in_maps = make_in_maps(X, flows, enc_params, dec_params, fa_params)
    res = run_bass_kernel_spmd(nc, in_maps, core_ids=list(range(NCORES)))
    return combine(res.results)
